# revision 49
# baseline (speedup 1.0000x reference)
"""Trainium2 Bass kernel for nn_AttentionBlock_38225208934579.

The reference attention block collapses algebraically: the scatter-sum
gathers v at edges_dst and scatters back to edges_dst, so for every
destination node d the attention weights (which sum to 1 over d's
segment) multiply the same vector v[d]:

    out[d] = x[d] + v[d] * [indegree(d) > 0],   v = norm_act(x @ Wv)

norm_act over 64x0e scalars is elementwise; with y = x @ Wv, u = |y|:

    v = sign(y) * (softplus(u) - log2)
      = y + w * sign(y),   w = ln(0.5*e^{-u} + 0.5)   (w in [-log2, 0])

so out = (x + y) + w*sign(y) — no division, no reciprocal needed.
The q/k/Wqk path of the reference is dead code.

Sharding: data parallel over nodes — each of the 8 cores handles 1024
nodes (8 graphs); the FxF weight is replicated.  Host-side prep packs
everything one core needs into a single [128, 1152] tensor (Wv once per
DMA-queue stream, a pair-interleaved transposed x so PE matmuls run as
h0/h64 row-group pairs with no on-device transposes, and x in SBUF
node-tile layout);
device output is the packed [128, 512] node-tile layout, unpacked on
the host.  Zero-indegree nodes (impossible for this problem's
block-diagonal fully-connected edges, where every node has 128
in-edges) keep x unchanged and are fixed up on the host.
"""

import math

import numpy as np

import concourse.mybir as mybir
import concourse.tile as tile
from concourse import bacc
from concourse.bass_utils import run_bass_kernel_spmd

N_NODES = 8192
F = 64
N_CORES = 8
NS = N_NODES // N_CORES  # 1024 nodes per core
NT = NS // 128           # 8 node-tiles of 128 per core
import os as _os
NCHUNK = int(_os.environ.get("K_NCHUNK", "2"))  # pipeline chunks (must be 2 or 4)
TPC = NT // NCHUNK       # tiles per chunk (4)
# Packed-input layout: [wvA | xt01 | wvB | xt23 | x03 | x47].  Wv appears
# once per HWDGE queue stream so every matmul gates on exactly one DMA
# semaphore (instructions hold a single wait slot).
WVA = 0
XTA = F                  # xt tile-pairs 0,1 (256 cols)
WVB = XTA + 2 * 128      # 320
XTB = WVB + F            # xt tile-pairs 2,3
X0 = XTB + 2 * 128       # 640: x tiles 0-7 (512 cols)
IN_W = X0 + NT * F       # 1152

AF = mybir.ActivationFunctionType
ALU = mybir.AluOpType

_cache: dict = {}

_ACT_SET = "natural_log_exp_and_others"


def _patch_act_tables():
    """bacc's table chooser greedily picks the lowest-index set containing
    each activation function, which splits {Abs, Exp} and {Ln} across two
    table loads (~2.7us each on the critical path).  Blank every set except
    the one that contains all of Abs/Exp/Ln/Copy so a single load is chosen.
    Positions are preserved, so the emitted act_func_set_id stays valid for
    walrus's lower_act."""
    if _cache.get("act_patched"):
        return
    real = bacc.get_activation_tables

    def only_full_set(arch):
        t = real(arch)
        if _ACT_SET in t:
            t = {k: (v if k == _ACT_SET else set()) for k, v in t.items()}
        return t

    bacc.get_activation_tables = only_full_set
    _cache["act_patched"] = True


def _build_bass():
    _patch_act_tables()
    nc = bacc.Bacc("TRN2", num_devices=N_CORES, enable_partition_id=False)
    in_d = nc.dram_tensor(
        "in_pack", (128, IN_W), mybir.dt.float32, kind="ExternalInput"
    ).ap()
    o_d = nc.dram_tensor(
        "out", (128, NT * F), mybir.dt.float32, kind="ExternalOutput"
    ).ap()


    with tile.TileContext(nc) as tc:
        with (
            tc.tile_pool(name="const", bufs=1) as cpool,
            tc.tile_pool(name="sb", bufs=1) as sb,
            tc.tile_pool(name="ew", bufs=2) as ew,
            tc.tile_pool(name="ps", bufs=1, space="PSUM") as ps,
        ):
            half = cpool.tile([128, 1], mybir.dt.float32)
            nc.gpsimd.memset(half[:], 0.5)

            o_sb = sb.tile([128, NT * F], mybir.dt.float32)
            in_sb = sb.tile([128, IN_W], mybir.dt.float32)
            # Four submits across BOTH HWDGE engines (Sync + Scalar) so two
            # hardware queues stream in parallel; Wv is duplicated into each
            # queue's stream and the splits sit on tile-pair boundaries, so
            # Tile's subtile deps gate each matmul on exactly one DMA.
            P1 = XTA + 128  # wvA + xt pair 0
            P3 = XTB + 128  # ... wvB + xt pair 2
            nc.sync.dma_start(in_sb[:, WVA:P1], in_d[:, WVA:P1])
            nc.scalar.dma_start(in_sb[:, WVB:P3], in_d[:, WVB:P3])
            nc.sync.dma_start(in_sb[:, P1:WVB], in_d[:, P1:WVB])
            nc.scalar.dma_start(in_sb[:, P3:X0], in_d[:, P3:X0])
            nc.sync.dma_start(in_sb[:, X0 : X0 + 4 * F], in_d[:, X0 : X0 + 4 * F])
            nc.scalar.dma_start(in_sb[:, X0 + 4 * F :], in_d[:, X0 + 4 * F :])

            # One PSUM tile where each node-tile's matmul owns a full bank
            # (two accumulation groups in one bank hang the PE), while a
            # single strided AP spans several banks for elementwise reads.
            y_full = ps.tile([128, NT, 512], mybir.dt.float32)
            y_ps = y_full[:, :, 0:F]

            CW = TPC * F  # chunk width in x/o columns (256)
            for c in range(NCHUNK):
                for t in range(c * TPC, (c + 1) * TPC):
                    i, h = t // 2, t % 2
                    xt_base = XTA + i * 128 if i < 2 else XTB + (i - 2) * 128
                    wv_base = WVA if i < 2 else WVB
                    nc.tensor.matmul(
                        y_ps[:, t],
                        in_sb[h * F : (h + 1) * F, xt_base : xt_base + 128],
                        in_sb[h * F : (h + 1) * F, wv_base : wv_base + F],
                        start=True,
                        stop=True,
                    )

                yc = y_ps[:, c * TPC : (c + 1) * TPC]          # [128, TPC, F]
                xc = in_sb[:, X0 + c * CW : X0 + (c + 1) * CW].rearrange(
                    "p (t c) -> p t c", t=TPC
                )
                oc = o_sb[:, c * CW : (c + 1) * CW].rearrange(
                    "p (t c) -> p t c", t=TPC
                )

                # Sign-free formulation: with g = ln(0.5*e^{-y} + 0.5)
                # (finite for |y| < 88) and s2 = (y >= 0)*2 in {2, 0},
                #   y >= 0: out = x + y + g
                #   y <  0: out = x - g
                # i.e. out = (x - g) + s2*(0.5*y + g).  Exp reads y straight
                # from PSUM (scale=-1), so ACT runs only Exp+Ln per chunk.
                e = ew.tile([128, CW], mybir.dt.float32)
                nc.scalar.activation(
                    e[:].rearrange("p (t c) -> p t c", t=TPC), yc, AF.Exp,
                    scale=-1.0,
                )
                g = ew.tile([128, CW], mybir.dt.float32)
                nc.scalar.activation(g[:], e[:], AF.Ln, bias=half[:], scale=0.5)
                s2 = ew.tile([128, CW], mybir.dt.float32)
                nc.vector.tensor_scalar(
                    s2[:].rearrange("p (t c) -> p t c", t=TPC),
                    yc, 0.0, 2.0, ALU.is_ge, ALU.mult,
                )
                t1 = ew.tile([128, CW], mybir.dt.float32)
                nc.vector.tensor_tensor(
                    t1[:].rearrange("p (t c) -> p t c", t=TPC), xc,
                    g[:].rearrange("p (t c) -> p t c", t=TPC), ALU.subtract,
                )
                t2 = ew.tile([128, CW], mybir.dt.float32)
                nc.vector.scalar_tensor_tensor(
                    t2[:].rearrange("p (t c) -> p t c", t=TPC),
                    yc, 0.5, g[:].rearrange("p (t c) -> p t c", t=TPC),
                    ALU.mult, ALU.add,
                )
                t3 = ew.tile([128, CW], mybir.dt.float32)
                nc.vector.tensor_tensor(t3[:], t2[:], s2[:], ALU.mult)
                nc.gpsimd.tensor_tensor(oc, t1[:].rearrange("p (t c) -> p t c", t=TPC), t3[:].rearrange("p (t c) -> p t c", t=TPC), ALU.add)
                # Two output submits per chunk, one per HWDGE engine, so both
                # queues drain the result in parallel.
                HW = CW // 2
                for h2 in range(2):
                    eng = nc.sync if h2 == 0 else nc.scalar
                    lo = c * CW + h2 * HW
                    eng.dma_start(o_d[:, lo : lo + HW], o_sb[:, lo : lo + HW])
    nc.compile()
    return nc


def _prep_core_inputs(x, Wv):
    """Pack each core's inputs into one [128, 1152] tensor:
    [wvA | xt01 | wvB | xt23 | x], matching the WVA/XTA/WVB/XTB/X0 layout.

    wv2: Wv stacked twice (h0/h64 matmul pairs need rhs at both base
    partitions); it appears once per HWDGE queue stream.
    xt[h*64+ch, 128*i + p] = xs[(2i+h)*128 + p, ch].
    xp[p, t*64+c] = xs[t*128 + p, c] (the SBUF node-tile layout).
    """
    wv2 = np.vstack([Wv, Wv])  # [128, F]
    maps = []
    for i in range(N_CORES):
        xs = x[i * NS : (i + 1) * NS]
        xt = xs.reshape(NT // 2, 2, 128, F).transpose(1, 3, 0, 2).reshape(128, NT * F)
        xp = xs.reshape(NT, 128, F).transpose(1, 0, 2).reshape(128, NT * F)
        maps.append(
            {
                "in_pack": np.ascontiguousarray(
                    np.hstack(
                        [wv2, xt[:, : 2 * 128], wv2, xt[:, 2 * 128 :], xp]
                    )
                )
            }
        )
    return maps


def _run(x, Wv, edges_dst, trace=False):
    x = np.ascontiguousarray(np.asarray(x, dtype=np.float32))
    Wv = np.ascontiguousarray(np.asarray(Wv, dtype=np.float32))
    if "nc" not in _cache:
        _cache["nc"] = _build_bass()
    nc = _cache["nc"]
    in_maps = _prep_core_inputs(x, Wv)
    res = run_bass_kernel_spmd(
        nc, in_maps, core_ids=list(range(N_CORES)), trace=trace
    )
    # Unpack [128, NT*F] node-tile layout back to [NS, F] per core.
    out = np.concatenate(
        [
            r["out"].reshape(128, NT, F).transpose(1, 0, 2).reshape(NS, F)
            for r in res.results
        ],
        axis=0,
    )
    # Residual-only rows: nodes with no incoming edge keep x unchanged.
    indeg = np.bincount(
        np.asarray(edges_dst).astype(np.int64), minlength=N_NODES
    )[:N_NODES]
    dead = indeg == 0
    if dead.any():
        out[dead] = x[dead]
    return out, res


def kernel(x, Wq, Wk, Wv, Wqk, edges_src, edges_dst):
    out, _ = _run(x, Wv, edges_dst)
    return out



# revision 52
# speedup vs baseline: 1.0087x; 1.0087x over previous
"""Trainium2 Bass kernel for nn_AttentionBlock_38225208934579.

The reference attention block collapses algebraically: the scatter-sum
gathers v at edges_dst and scatters back to edges_dst, so for every
destination node d the attention weights (which sum to 1 over d's
segment) multiply the same vector v[d]:

    out[d] = x[d] + v[d] * [indegree(d) > 0],   v = norm_act(x @ Wv)

norm_act over 64x0e scalars is elementwise; with y = x @ Wv, u = |y|:

    v = sign(y) * (softplus(u) - log2)
      = y + w * sign(y),   w = ln(0.5*e^{-u} + 0.5)   (w in [-log2, 0])

so out = (x + y) + w*sign(y) — no division, no reciprocal needed.
The q/k/Wqk path of the reference is dead code.

Sharding: data parallel over nodes — each of the 8 cores handles 1024
nodes (8 graphs); the FxF weight is replicated.  Host-side prep packs
everything one core needs into a single [128, 1152] tensor (Wv once per
DMA-queue stream, a pair-interleaved transposed x so PE matmuls run as
h0/h64 row-group pairs with no on-device transposes, and x in SBUF
node-tile layout);
device output is the packed [128, 512] node-tile layout, unpacked on
the host.  Zero-indegree nodes (impossible for this problem's
block-diagonal fully-connected edges, where every node has 128
in-edges) keep x unchanged and are fixed up on the host.
"""

import math

import numpy as np

import concourse.mybir as mybir
import concourse.tile as tile
from concourse import bacc
from concourse.bass_utils import run_bass_kernel_spmd

N_NODES = 8192
F = 64
N_CORES = 8
NS = N_NODES // N_CORES  # 1024 nodes per core
NT = NS // 128           # 8 node-tiles of 128 per core
import os as _os
NCHUNK = int(_os.environ.get("K_NCHUNK", "2"))  # pipeline chunks (must be 2 or 4)
TPC = NT // NCHUNK       # tiles per chunk (4)
# Packed-input layout: [wvA | xt01 | wvB | xt23 | x03 | x47].  Wv appears
# once per HWDGE queue stream so every matmul gates on exactly one DMA
# semaphore (instructions hold a single wait slot).
WVA = 0
XTA = F                  # xt tile-pairs 0,1 (256 cols)
WVB = XTA + 2 * 128      # 320
XTB = WVB + F            # xt tile-pairs 2,3
X0 = XTB + 2 * 128       # 640: x tiles 0-7 (512 cols)
IN_W = X0 + NT * F       # 1152

AF = mybir.ActivationFunctionType
ALU = mybir.AluOpType

_cache: dict = {}

_ACT_SET = "natural_log_exp_and_others"


def _patch_act_tables():
    """bacc's table chooser greedily picks the lowest-index set containing
    each activation function, which splits {Abs, Exp} and {Ln} across two
    table loads (~2.7us each on the critical path).  Blank every set except
    the one that contains all of Abs/Exp/Ln/Copy so a single load is chosen.
    Positions are preserved, so the emitted act_func_set_id stays valid for
    walrus's lower_act."""
    if _cache.get("act_patched"):
        return
    real = bacc.get_activation_tables

    def only_full_set(arch):
        t = real(arch)
        if _ACT_SET in t:
            t = {k: (v if k == _ACT_SET else set()) for k, v in t.items()}
        return t

    bacc.get_activation_tables = only_full_set
    _cache["act_patched"] = True


def _build_bass():
    _patch_act_tables()
    nc = bacc.Bacc("TRN2", num_devices=N_CORES, enable_partition_id=False)
    in_d = nc.dram_tensor(
        "in_pack", (128, IN_W), mybir.dt.float32, kind="ExternalInput"
    ).ap()
    o_d = nc.dram_tensor(
        "out", (128, NT * F), mybir.dt.float32, kind="ExternalOutput"
    ).ap()


    with tile.TileContext(nc) as tc:
        with (
            tc.tile_pool(name="const", bufs=1) as cpool,
            tc.tile_pool(name="sb", bufs=1) as sb,
            tc.tile_pool(name="ew", bufs=2) as ew,
            tc.tile_pool(name="ps", bufs=1, space="PSUM") as ps,
        ):
            half = cpool.tile([128, 1], mybir.dt.float32)
            nc.gpsimd.memset(half[:], 0.5)

            o_sb = sb.tile([128, NT * F], mybir.dt.float32)
            in_sb = sb.tile([128, IN_W], mybir.dt.float32)
            # Four submits across BOTH HWDGE engines (Sync + Scalar) so two
            # hardware queues stream in parallel; Wv is duplicated into each
            # queue's stream and the splits sit on tile-pair boundaries, so
            # Tile's subtile deps gate each matmul on exactly one DMA.
            P1 = XTA + 128  # wvA + xt pair 0
            P3 = XTB + 128  # ... wvB + xt pair 2
            nc.sync.dma_start(in_sb[:, WVA:P1], in_d[:, WVA:P1])
            nc.scalar.dma_start(in_sb[:, WVB:P3], in_d[:, WVB:P3])
            nc.sync.dma_start(in_sb[:, P1:WVB], in_d[:, P1:WVB])
            nc.scalar.dma_start(in_sb[:, P3:X0], in_d[:, P3:X0])
            nc.sync.dma_start(in_sb[:, X0 : X0 + 4 * F], in_d[:, X0 : X0 + 4 * F])
            nc.scalar.dma_start(in_sb[:, X0 + 4 * F :], in_d[:, X0 + 4 * F :])

            # One PSUM tile where each node-tile's matmul owns a full bank
            # (two accumulation groups in one bank hang the PE), while a
            # single strided AP spans several banks for elementwise reads.
            y_full = ps.tile([128, NT, 512], mybir.dt.float32)
            y_ps = y_full[:, :, 0:F]

            # Uneven chunks: a small LAST chunk shortens the tail (its
            # elementwise chain and final output DMA gate the kernel end).
            bounds = (
                [(0, 6), (6, 8)]
                if NCHUNK == 2
                else [(i * TPC, (i + 1) * TPC) for i in range(NCHUNK)]
            )
            for lo_t, hi_t in bounds:
                TC_ = hi_t - lo_t
                CW = TC_ * F
                C0 = lo_t * F
                for t in range(lo_t, hi_t):
                    i, h = t // 2, t % 2
                    xt_base = XTA + i * 128 if i < 2 else XTB + (i - 2) * 128
                    wv_base = WVA if i < 2 else WVB
                    nc.tensor.matmul(
                        y_ps[:, t],
                        in_sb[h * F : (h + 1) * F, xt_base : xt_base + 128],
                        in_sb[h * F : (h + 1) * F, wv_base : wv_base + F],
                        start=True,
                        stop=True,
                    )

                yc = y_ps[:, lo_t:hi_t]                        # [128, TC_, F]
                xc = in_sb[:, X0 + C0 : X0 + C0 + CW].rearrange(
                    "p (t c) -> p t c", t=TC_
                )
                oc = o_sb[:, C0 : C0 + CW].rearrange("p (t c) -> p t c", t=TC_)

                # Sign-free formulation: with g = ln(0.5*e^{-y} + 0.5)
                # (finite for |y| < 88) and s2 = (y >= 0)*2 in {2, 0},
                #   y >= 0: out = x + y + g
                #   y <  0: out = x - g
                # i.e. out = (x - g) + s2*(0.5*y + g).  Exp reads y straight
                # from PSUM (scale=-1), so ACT runs only Exp+Ln per chunk.
                e = ew.tile([128, CW], mybir.dt.float32)
                nc.scalar.activation(
                    e[:].rearrange("p (t c) -> p t c", t=TC_), yc, AF.Exp,
                    scale=-1.0,
                )
                g = ew.tile([128, CW], mybir.dt.float32)
                nc.scalar.activation(g[:], e[:], AF.Ln, bias=half[:], scale=0.5)
                s2 = ew.tile([128, CW], mybir.dt.float32)
                nc.vector.tensor_scalar(
                    s2[:].rearrange("p (t c) -> p t c", t=TC_),
                    yc, 0.0, 2.0, ALU.is_ge, ALU.mult,
                )
                t1 = ew.tile([128, CW], mybir.dt.float32)
                nc.vector.tensor_tensor(
                    t1[:].rearrange("p (t c) -> p t c", t=TC_), xc,
                    g[:].rearrange("p (t c) -> p t c", t=TC_), ALU.subtract,
                )
                t2 = ew.tile([128, CW], mybir.dt.float32)
                nc.vector.scalar_tensor_tensor(
                    t2[:].rearrange("p (t c) -> p t c", t=TC_),
                    yc, 0.5, g[:].rearrange("p (t c) -> p t c", t=TC_),
                    ALU.mult, ALU.add,
                )
                t3 = ew.tile([128, CW], mybir.dt.float32)
                nc.vector.tensor_tensor(t3[:], t2[:], s2[:], ALU.mult)
                nc.gpsimd.tensor_tensor(oc, t1[:].rearrange("p (t c) -> p t c", t=TC_), t3[:].rearrange("p (t c) -> p t c", t=TC_), ALU.add)
                # Two output submits per chunk, one per HWDGE engine, so both
                # queues drain the result in parallel.
                HW = CW // 2
                for h2 in range(2):
                    eng = nc.sync if h2 == 0 else nc.scalar
                    lo = C0 + h2 * HW
                    eng.dma_start(o_d[:, lo : lo + HW], o_sb[:, lo : lo + HW])
    nc.compile()
    return nc


def _prep_core_inputs(x, Wv):
    """Pack each core's inputs into one [128, 1152] tensor:
    [wvA | xt01 | wvB | xt23 | x], matching the WVA/XTA/WVB/XTB/X0 layout.

    wv2: Wv stacked twice (h0/h64 matmul pairs need rhs at both base
    partitions); it appears once per HWDGE queue stream.
    xt[h*64+ch, 128*i + p] = xs[(2i+h)*128 + p, ch].
    xp[p, t*64+c] = xs[t*128 + p, c] (the SBUF node-tile layout).
    """
    wv2 = np.vstack([Wv, Wv])  # [128, F]
    maps = []
    for i in range(N_CORES):
        xs = x[i * NS : (i + 1) * NS]
        xt = xs.reshape(NT // 2, 2, 128, F).transpose(1, 3, 0, 2).reshape(128, NT * F)
        xp = xs.reshape(NT, 128, F).transpose(1, 0, 2).reshape(128, NT * F)
        maps.append(
            {
                "in_pack": np.ascontiguousarray(
                    np.hstack(
                        [wv2, xt[:, : 2 * 128], wv2, xt[:, 2 * 128 :], xp]
                    )
                )
            }
        )
    return maps


def _run(x, Wv, edges_dst, trace=False):
    x = np.ascontiguousarray(np.asarray(x, dtype=np.float32))
    Wv = np.ascontiguousarray(np.asarray(Wv, dtype=np.float32))
    if "nc" not in _cache:
        _cache["nc"] = _build_bass()
    nc = _cache["nc"]
    in_maps = _prep_core_inputs(x, Wv)
    res = run_bass_kernel_spmd(
        nc, in_maps, core_ids=list(range(N_CORES)), trace=trace
    )
    # Unpack [128, NT*F] node-tile layout back to [NS, F] per core.
    out = np.concatenate(
        [
            r["out"].reshape(128, NT, F).transpose(1, 0, 2).reshape(NS, F)
            for r in res.results
        ],
        axis=0,
    )
    # Residual-only rows: nodes with no incoming edge keep x unchanged.
    indeg = np.bincount(
        np.asarray(edges_dst).astype(np.int64), minlength=N_NODES
    )[:N_NODES]
    dead = indeg == 0
    if dead.any():
        out[dead] = x[dead]
    return out, res


def kernel(x, Wq, Wk, Wv, Wqk, edges_src, edges_dst):
    out, _ = _run(x, Wv, edges_dst)
    return out



# revision 53
# speedup vs baseline: 1.0122x; 1.0035x over previous
"""Trainium2 Bass kernel for nn_AttentionBlock_38225208934579.

The reference attention block collapses algebraically: the scatter-sum
gathers v at edges_dst and scatters back to edges_dst, so for every
destination node d the attention weights (which sum to 1 over d's
segment) multiply the same vector v[d]:

    out[d] = x[d] + v[d] * [indegree(d) > 0],   v = norm_act(x @ Wv)

norm_act over 64x0e scalars is elementwise; with y = x @ Wv, u = |y|:

    v = sign(y) * (softplus(u) - log2)
      = y + w * sign(y),   w = ln(0.5*e^{-u} + 0.5)   (w in [-log2, 0])

so out = (x + y) + w*sign(y) — no division, no reciprocal needed.
The q/k/Wqk path of the reference is dead code.

Sharding: data parallel over nodes — each of the 8 cores handles 1024
nodes (8 graphs); the FxF weight is replicated.  Host-side prep packs
everything one core needs into a single [128, 1152] tensor (Wv once per
DMA-queue stream, a pair-interleaved transposed x so PE matmuls run as
h0/h64 row-group pairs with no on-device transposes, and x in SBUF
node-tile layout);
device output is the packed [128, 512] node-tile layout, unpacked on
the host.  Zero-indegree nodes (impossible for this problem's
block-diagonal fully-connected edges, where every node has 128
in-edges) keep x unchanged and are fixed up on the host.
"""

import math

import numpy as np

import concourse.mybir as mybir
import concourse.tile as tile
from concourse import bacc
from concourse.bass_utils import run_bass_kernel_spmd

N_NODES = 8192
F = 64
N_CORES = 8
NS = N_NODES // N_CORES  # 1024 nodes per core
NT = NS // 128           # 8 node-tiles of 128 per core
import os as _os
NCHUNK = int(_os.environ.get("K_NCHUNK", "2"))  # pipeline chunks (must be 2 or 4)
TPC = NT // NCHUNK       # tiles per chunk (4)
# Packed-input layout: [wvA | xt01 | wvB | xt23 | x03 | x47].  Wv appears
# once per HWDGE queue stream so every matmul gates on exactly one DMA
# semaphore (instructions hold a single wait slot).
WVA = 0
XTA = F                  # xt tile-pairs 0,1 (256 cols)
WVB = XTA + 2 * 128      # 320
XTB = WVB + F            # xt tile-pairs 2,3
X0 = XTB + 2 * 128       # 640: x tiles 0-7 (512 cols)
IN_W = X0 + NT * F       # 1152

AF = mybir.ActivationFunctionType
ALU = mybir.AluOpType

_cache: dict = {}

_ACT_SET = "natural_log_exp_and_others"


def _patch_act_tables():
    """bacc's table chooser greedily picks the lowest-index set containing
    each activation function, which splits {Abs, Exp} and {Ln} across two
    table loads (~2.7us each on the critical path).  Blank every set except
    the one that contains all of Abs/Exp/Ln/Copy so a single load is chosen.
    Positions are preserved, so the emitted act_func_set_id stays valid for
    walrus's lower_act."""
    if _cache.get("act_patched"):
        return
    real = bacc.get_activation_tables

    def only_full_set(arch):
        t = real(arch)
        if _ACT_SET in t:
            t = {k: (v if k == _ACT_SET else set()) for k, v in t.items()}
        return t

    bacc.get_activation_tables = only_full_set
    _cache["act_patched"] = True


def _build_bass():
    _patch_act_tables()
    nc = bacc.Bacc("TRN2", num_devices=N_CORES, enable_partition_id=False)
    in_d = nc.dram_tensor(
        "in_pack", (128, IN_W), mybir.dt.float32, kind="ExternalInput"
    ).ap()
    o_d = nc.dram_tensor(
        "out", (128, NT * F), mybir.dt.float32, kind="ExternalOutput"
    ).ap()


    with tile.TileContext(nc) as tc:
        with (
            tc.tile_pool(name="const", bufs=1) as cpool,
            tc.tile_pool(name="sb", bufs=1) as sb,
            tc.tile_pool(name="ew", bufs=2) as ew,
            tc.tile_pool(name="ps", bufs=1, space="PSUM") as ps,
        ):
            half = cpool.tile([128, 1], mybir.dt.float32)
            nc.gpsimd.memset(half[:], 0.5)

            o_sb = sb.tile([128, NT * F], mybir.dt.float32)
            in_sb = sb.tile([128, IN_W], mybir.dt.float32)
            # Four submits across BOTH HWDGE engines (Sync + Scalar) so two
            # hardware queues stream in parallel; Wv is duplicated into each
            # queue's stream and the splits sit on tile-pair boundaries, so
            # Tile's subtile deps gate each matmul on exactly one DMA.
            P1 = XTA + 128  # wvA + xt pair 0
            P3 = XTB + 128  # ... wvB + xt pair 2
            nc.sync.dma_start(in_sb[:, WVA:P1], in_d[:, WVA:P1])
            nc.scalar.dma_start(in_sb[:, WVB:P3], in_d[:, WVB:P3])
            nc.sync.dma_start(in_sb[:, P1:WVB], in_d[:, P1:WVB])
            nc.scalar.dma_start(in_sb[:, P3:X0], in_d[:, P3:X0])
            nc.sync.dma_start(in_sb[:, X0 : X0 + 4 * F], in_d[:, X0 : X0 + 4 * F])
            nc.scalar.dma_start(in_sb[:, X0 + 4 * F :], in_d[:, X0 + 4 * F :])

            # One PSUM tile where each node-tile's matmul owns a full bank
            # (two accumulation groups in one bank hang the PE), while a
            # single strided AP spans several banks for elementwise reads.
            y_full = ps.tile([128, NT, 512], mybir.dt.float32)
            y_ps = y_full[:, :, 0:F]

            bounds = [(i * TPC, (i + 1) * TPC) for i in range(NCHUNK)]
            for lo_t, hi_t in bounds:
                TC_ = hi_t - lo_t
                CW = TC_ * F
                C0 = lo_t * F
                for t in range(lo_t, hi_t):
                    i, h = t // 2, t % 2
                    xt_base = XTA + i * 128 if i < 2 else XTB + (i - 2) * 128
                    wv_base = WVA if i < 2 else WVB
                    nc.tensor.matmul(
                        y_ps[:, t],
                        in_sb[h * F : (h + 1) * F, xt_base : xt_base + 128],
                        in_sb[h * F : (h + 1) * F, wv_base : wv_base + F],
                        start=True,
                        stop=True,
                    )

                yc = y_ps[:, lo_t:hi_t]                        # [128, TC_, F]
                xc = in_sb[:, X0 + C0 : X0 + C0 + CW].rearrange(
                    "p (t c) -> p t c", t=TC_
                )
                oc = o_sb[:, C0 : C0 + CW].rearrange("p (t c) -> p t c", t=TC_)

                # Sign-free formulation: with g = ln(0.5*e^{-y} + 0.5)
                # (finite for |y| < 88) and s2 = (y >= 0)*2 in {2, 0},
                #   y >= 0: out = x + y + g
                #   y <  0: out = x - g
                # i.e. out = (x - g) + s2*(0.5*y + g).  Exp reads y straight
                # from PSUM (scale=-1), so ACT runs only Exp+Ln per chunk.
                e = ew.tile([128, CW], mybir.dt.float32)
                nc.scalar.activation(
                    e[:].rearrange("p (t c) -> p t c", t=TC_), yc, AF.Exp,
                    scale=-1.0,
                )
                g = ew.tile([128, CW], mybir.dt.float32)
                nc.scalar.activation(g[:], e[:], AF.Ln, bias=half[:], scale=0.5)
                s2 = ew.tile([128, CW], mybir.dt.float32)
                nc.vector.tensor_scalar(
                    s2[:].rearrange("p (t c) -> p t c", t=TC_),
                    yc, 0.0, 2.0, ALU.is_ge, ALU.mult,
                )
                t1 = ew.tile([128, CW], mybir.dt.float32)
                nc.vector.tensor_tensor(
                    t1[:].rearrange("p (t c) -> p t c", t=TC_), xc,
                    g[:].rearrange("p (t c) -> p t c", t=TC_), ALU.subtract,
                )
                t2 = ew.tile([128, CW], mybir.dt.float32)
                nc.vector.scalar_tensor_tensor(
                    t2[:].rearrange("p (t c) -> p t c", t=TC_),
                    yc, 0.5, g[:].rearrange("p (t c) -> p t c", t=TC_),
                    ALU.mult, ALU.add,
                )
                t3 = ew.tile([128, CW], mybir.dt.float32)
                nc.vector.tensor_tensor(t3[:], t2[:], s2[:], ALU.mult)
                nc.gpsimd.tensor_tensor(oc, t1[:].rearrange("p (t c) -> p t c", t=TC_), t3[:].rearrange("p (t c) -> p t c", t=TC_), ALU.add)
                # Two output submits per chunk, one per HWDGE engine, so both
                # queues drain the result in parallel.
                HW = CW // 2
                for h2 in range(2):
                    eng = nc.sync if h2 == 0 else nc.scalar
                    lo = C0 + h2 * HW
                    eng.dma_start(o_d[:, lo : lo + HW], o_sb[:, lo : lo + HW])
    nc.compile()
    return nc


def _prep_core_inputs(x, Wv):
    """Pack each core's inputs into one [128, 1152] tensor:
    [wvA | xt01 | wvB | xt23 | x], matching the WVA/XTA/WVB/XTB/X0 layout.

    wv2: Wv stacked twice (h0/h64 matmul pairs need rhs at both base
    partitions); it appears once per HWDGE queue stream.
    xt[h*64+ch, 128*i + p] = xs[(2i+h)*128 + p, ch].
    xp[p, t*64+c] = xs[t*128 + p, c] (the SBUF node-tile layout).
    """
    wv2 = np.vstack([Wv, Wv])  # [128, F]
    maps = []
    for i in range(N_CORES):
        xs = x[i * NS : (i + 1) * NS]
        xt = xs.reshape(NT // 2, 2, 128, F).transpose(1, 3, 0, 2).reshape(128, NT * F)
        xp = xs.reshape(NT, 128, F).transpose(1, 0, 2).reshape(128, NT * F)
        maps.append(
            {
                "in_pack": np.ascontiguousarray(
                    np.hstack(
                        [wv2, xt[:, : 2 * 128], wv2, xt[:, 2 * 128 :], xp]
                    )
                )
            }
        )
    return maps


def _run(x, Wv, edges_dst, trace=False):
    x = np.ascontiguousarray(np.asarray(x, dtype=np.float32))
    Wv = np.ascontiguousarray(np.asarray(Wv, dtype=np.float32))
    if "nc" not in _cache:
        _cache["nc"] = _build_bass()
    nc = _cache["nc"]
    in_maps = _prep_core_inputs(x, Wv)
    res = run_bass_kernel_spmd(
        nc, in_maps, core_ids=list(range(N_CORES)), trace=trace
    )
    # Unpack [128, NT*F] node-tile layout back to [NS, F] per core.
    out = np.concatenate(
        [
            r["out"].reshape(128, NT, F).transpose(1, 0, 2).reshape(NS, F)
            for r in res.results
        ],
        axis=0,
    )
    # Residual-only rows: nodes with no incoming edge keep x unchanged.
    indeg = np.bincount(
        np.asarray(edges_dst).astype(np.int64), minlength=N_NODES
    )[:N_NODES]
    dead = indeg == 0
    if dead.any():
        out[dead] = x[dead]
    return out, res


def kernel(x, Wq, Wk, Wv, Wqk, edges_src, edges_dst):
    out, _ = _run(x, Wv, edges_dst)
    return out



# revision 56
# speedup vs baseline: 1.0184x; 1.0061x over previous
"""Trainium2 Bass kernel for nn_AttentionBlock_38225208934579.

The reference attention block collapses algebraically: the scatter-sum
gathers v at edges_dst and scatters back to edges_dst, so for every
destination node d the attention weights (which sum to 1 over d's
segment) multiply the same vector v[d]:

    out[d] = x[d] + v[d] * [indegree(d) > 0],   v = norm_act(x @ Wv)

norm_act over 64x0e scalars is elementwise; with y = x @ Wv, u = |y|:

    v = sign(y) * (softplus(u) - log2)
      = y + w * sign(y),   w = ln(0.5*e^{-u} + 0.5)   (w in [-log2, 0])

so out = (x + y) + w*sign(y) — no division, no reciprocal needed.
The q/k/Wqk path of the reference is dead code.

Sharding: data parallel over nodes — each of the 8 cores handles 1024
nodes (8 graphs); the FxF weight is replicated.  Host-side prep packs
everything one core needs into a single [128, 1152] tensor (Wv once per
DMA-queue stream, a pair-interleaved transposed x so PE matmuls run as
h0/h64 row-group pairs with no on-device transposes, and x in SBUF
node-tile layout);
device output is the packed [128, 512] node-tile layout, unpacked on
the host.  Zero-indegree nodes (impossible for this problem's
block-diagonal fully-connected edges, where every node has 128
in-edges) keep x unchanged and are fixed up on the host.
"""

import math

import numpy as np

import concourse.mybir as mybir
import concourse.tile as tile
from concourse import bacc
from concourse.bass_utils import run_bass_kernel_spmd

N_NODES = 8192
F = 64
N_CORES = 8
NS = N_NODES // N_CORES  # 1024 nodes per core
NT = NS // 128           # 8 node-tiles of 128 per core
import os as _os
NCHUNK = int(_os.environ.get("K_NCHUNK", "2"))  # pipeline chunks (must be 2 or 4)
TPC = NT // NCHUNK       # tiles per chunk (4)
# Packed-input layout: [wvA | xt01 | wvB | xt23 | x03 | x47].  Wv appears
# once per HWDGE queue stream so every matmul gates on exactly one DMA
# semaphore (instructions hold a single wait slot).
WVA = 0
XTA = F                  # xt tile-pairs 0,1 (256 cols)
WVB = XTA + 2 * 128      # 320
XTB = WVB + F            # xt tile-pairs 2,3
X0 = XTB + 2 * 128       # 640: x tiles 0-7 (512 cols)
IN_W = X0 + NT * F       # 1152

AF = mybir.ActivationFunctionType
ALU = mybir.AluOpType

_cache: dict = {}

_ACT_SET = "natural_log_exp_and_others"


def _patch_act_tables():
    """bacc's table chooser greedily picks the lowest-index set containing
    each activation function, which splits {Abs, Exp} and {Ln} across two
    table loads (~2.7us each on the critical path).  Blank every set except
    the one that contains all of Abs/Exp/Ln/Copy so a single load is chosen.
    Positions are preserved, so the emitted act_func_set_id stays valid for
    walrus's lower_act."""
    if _cache.get("act_patched"):
        return
    real = bacc.get_activation_tables

    def only_full_set(arch):
        t = real(arch)
        if _ACT_SET in t:
            t = {k: (v if k == _ACT_SET else set()) for k, v in t.items()}
        return t

    bacc.get_activation_tables = only_full_set
    _cache["act_patched"] = True


def _build_bass():
    _patch_act_tables()
    nc = bacc.Bacc("TRN2", num_devices=N_CORES, enable_partition_id=False)
    in_d = nc.dram_tensor(
        "in_pack", (128, IN_W), mybir.dt.float32, kind="ExternalInput"
    ).ap()
    o_d = nc.dram_tensor(
        "out", (128, NT * F), mybir.dt.float32, kind="ExternalOutput"
    ).ap()


    with tile.TileContext(nc) as tc:
        with (
            tc.tile_pool(name="const", bufs=1) as cpool,
            tc.tile_pool(name="sb", bufs=1) as sb,
            tc.tile_pool(name="ew", bufs=2) as ew,
            tc.tile_pool(name="ps", bufs=1, space="PSUM") as ps,
        ):
            half = cpool.tile([128, 1], mybir.dt.float32)
            nc.gpsimd.memset(half[:], 0.5)

            o_sb = sb.tile([128, NT * F], mybir.dt.float32)
            in_sb = sb.tile([128, IN_W], mybir.dt.float32)
            # Four submits across BOTH HWDGE engines (Sync + Scalar) so two
            # hardware queues stream in parallel; Wv is duplicated into each
            # queue's stream and the splits sit on tile-pair boundaries, so
            # Tile's subtile deps gate each matmul on exactly one DMA.
            P1 = XTA + 128  # wvA + xt pair 0
            P3 = XTB + 128  # ... wvB + xt pair 2
            nc.sync.dma_start(in_sb[:, WVA:P1], in_d[:, WVA:P1])
            nc.scalar.dma_start(in_sb[:, WVB:P3], in_d[:, WVB:P3])
            nc.sync.dma_start(in_sb[:, P1:WVB], in_d[:, P1:WVB])
            nc.scalar.dma_start(in_sb[:, P3:X0], in_d[:, P3:X0])
            nc.sync.dma_start(in_sb[:, X0 : X0 + 4 * F], in_d[:, X0 : X0 + 4 * F])
            nc.scalar.dma_start(in_sb[:, X0 + 4 * F :], in_d[:, X0 + 4 * F :])

            # One PSUM tile where each node-tile's matmul owns a full bank
            # (two accumulation groups in one bank hang the PE), while a
            # single strided AP spans several banks for elementwise reads.
            y_full = ps.tile([128, NT, 512], mybir.dt.float32)
            y_ps = y_full[:, :, 0:F]

            bounds = [(i * TPC, (i + 1) * TPC) for i in range(NCHUNK)]
            for lo_t, hi_t in bounds:
                TC_ = hi_t - lo_t
                CW = TC_ * F
                C0 = lo_t * F
                for t in range(lo_t, hi_t):
                    i, h = t // 2, t % 2
                    xt_base = XTA + i * 128 if i < 2 else XTB + (i - 2) * 128
                    wv_base = WVA if i < 2 else WVB
                    nc.tensor.matmul(
                        y_ps[:, t],
                        in_sb[h * F : (h + 1) * F, xt_base : xt_base + 128],
                        in_sb[h * F : (h + 1) * F, wv_base : wv_base + F],
                        start=True,
                        stop=True,
                    )

                yc = y_ps[:, lo_t:hi_t]                        # [128, TC_, F]
                xc = in_sb[:, X0 + C0 : X0 + C0 + CW].rearrange(
                    "p (t c) -> p t c", t=TC_
                )
                oc = o_sb[:, C0 : C0 + CW].rearrange("p (t c) -> p t c", t=TC_)

                # With g = ln(0.5*e^{-y} + 0.5) (finite for |y| < 88)
                # and s = sign(y):
                #   y >= 0: out = x + y + g;   y < 0: out = x - g
                # i.e. out = (x - g) + (s+1)*(0.5*y + g), exact at y=0 too.
                # Exp reads y straight from PSUM (scale=-1); ACT also takes
                # Sign so the DVE stream stays short.
                e = ew.tile([128, CW], mybir.dt.float32)
                nc.scalar.activation(
                    e[:].rearrange("p (t c) -> p t c", t=TC_), yc, AF.Exp,
                    scale=-1.0,
                )
                g = ew.tile([128, CW], mybir.dt.float32)
                nc.scalar.activation(g[:], e[:], AF.Ln, bias=half[:], scale=0.5)
                s2 = ew.tile([128, CW], mybir.dt.float32)
                nc.scalar.activation(
                    s2[:].rearrange("p (t c) -> p t c", t=TC_), yc, AF.Sign
                )
                t1 = ew.tile([128, CW], mybir.dt.float32)
                nc.vector.tensor_tensor(
                    t1[:].rearrange("p (t c) -> p t c", t=TC_), xc,
                    g[:].rearrange("p (t c) -> p t c", t=TC_), ALU.subtract,
                )
                t2 = ew.tile([128, CW], mybir.dt.float32)
                nc.vector.scalar_tensor_tensor(
                    t2[:].rearrange("p (t c) -> p t c", t=TC_),
                    yc, 0.5, g[:].rearrange("p (t c) -> p t c", t=TC_),
                    ALU.mult, ALU.add,
                )
                t3 = ew.tile([128, CW], mybir.dt.float32)
                nc.vector.scalar_tensor_tensor(
                    t3[:], s2[:], 1.0, t2[:], ALU.add, ALU.mult
                )
                nc.vector.tensor_tensor(oc, t1[:].rearrange("p (t c) -> p t c", t=TC_), t3[:].rearrange("p (t c) -> p t c", t=TC_), ALU.add)
                # Two output submits per chunk, one per HWDGE engine, so both
                # queues drain the result in parallel.
                HW = CW // 2
                for h2 in range(2):
                    eng = nc.sync if h2 == 0 else nc.scalar
                    lo = C0 + h2 * HW
                    eng.dma_start(o_d[:, lo : lo + HW], o_sb[:, lo : lo + HW])
    nc.compile()
    return nc


def _prep_core_inputs(x, Wv):
    """Pack each core's inputs into one [128, 1152] tensor:
    [wvA | xt01 | wvB | xt23 | x], matching the WVA/XTA/WVB/XTB/X0 layout.

    wv2: Wv stacked twice (h0/h64 matmul pairs need rhs at both base
    partitions); it appears once per HWDGE queue stream.
    xt[h*64+ch, 128*i + p] = xs[(2i+h)*128 + p, ch].
    xp[p, t*64+c] = xs[t*128 + p, c] (the SBUF node-tile layout).
    """
    wv2 = np.vstack([Wv, Wv])  # [128, F]
    maps = []
    for i in range(N_CORES):
        xs = x[i * NS : (i + 1) * NS]
        xt = xs.reshape(NT // 2, 2, 128, F).transpose(1, 3, 0, 2).reshape(128, NT * F)
        xp = xs.reshape(NT, 128, F).transpose(1, 0, 2).reshape(128, NT * F)
        maps.append(
            {
                "in_pack": np.ascontiguousarray(
                    np.hstack(
                        [wv2, xt[:, : 2 * 128], wv2, xt[:, 2 * 128 :], xp]
                    )
                )
            }
        )
    return maps


def _run(x, Wv, edges_dst, trace=False):
    x = np.ascontiguousarray(np.asarray(x, dtype=np.float32))
    Wv = np.ascontiguousarray(np.asarray(Wv, dtype=np.float32))
    if "nc" not in _cache:
        _cache["nc"] = _build_bass()
    nc = _cache["nc"]
    in_maps = _prep_core_inputs(x, Wv)
    res = run_bass_kernel_spmd(
        nc, in_maps, core_ids=list(range(N_CORES)), trace=trace
    )
    # Unpack [128, NT*F] node-tile layout back to [NS, F] per core.
    out = np.concatenate(
        [
            r["out"].reshape(128, NT, F).transpose(1, 0, 2).reshape(NS, F)
            for r in res.results
        ],
        axis=0,
    )
    # Residual-only rows: nodes with no incoming edge keep x unchanged.
    indeg = np.bincount(
        np.asarray(edges_dst).astype(np.int64), minlength=N_NODES
    )[:N_NODES]
    dead = indeg == 0
    if dead.any():
        out[dead] = x[dead]
    return out, res


def kernel(x, Wq, Wk, Wv, Wqk, edges_src, edges_dst):
    out, _ = _run(x, Wv, edges_dst)
    return out



# revision 57
# speedup vs baseline: 1.0271x; 1.0085x over previous
"""Trainium2 Bass kernel for nn_AttentionBlock_38225208934579.

The reference attention block collapses algebraically: the scatter-sum
gathers v at edges_dst and scatters back to edges_dst, so for every
destination node d the attention weights (which sum to 1 over d's
segment) multiply the same vector v[d]:

    out[d] = x[d] + v[d] * [indegree(d) > 0],   v = norm_act(x @ Wv)

norm_act over 64x0e scalars is elementwise; with y = x @ Wv, u = |y|:

    v = sign(y) * (softplus(u) - log2)
      = y + w * sign(y),   w = ln(0.5*e^{-u} + 0.5)   (w in [-log2, 0])

so out = (x + y) + w*sign(y) — no division, no reciprocal needed.
The q/k/Wqk path of the reference is dead code.

Sharding: data parallel over nodes — each of the 8 cores handles 1024
nodes (8 graphs); the FxF weight is replicated.  Host-side prep packs
everything one core needs into a single [128, 1152] tensor (Wv once per
DMA-queue stream, a pair-interleaved transposed x so PE matmuls run as
h0/h64 row-group pairs with no on-device transposes, and x in SBUF
node-tile layout);
device output is the packed [128, 512] node-tile layout, unpacked on
the host.  Zero-indegree nodes (impossible for this problem's
block-diagonal fully-connected edges, where every node has 128
in-edges) keep x unchanged and are fixed up on the host.
"""

import math

import numpy as np

import concourse.mybir as mybir
import concourse.tile as tile
from concourse import bacc
from concourse.bass_utils import run_bass_kernel_spmd

N_NODES = 8192
F = 64
N_CORES = 8
NS = N_NODES // N_CORES  # 1024 nodes per core
NT = NS // 128           # 8 node-tiles of 128 per core
import os as _os
NCHUNK = int(_os.environ.get("K_NCHUNK", "2"))  # pipeline chunks (must be 2 or 4)
TPC = NT // NCHUNK       # tiles per chunk (4)
# Packed-input layout: [wvA | xt01 | wvB | xt23 | x03 | x47].  Wv appears
# once per HWDGE queue stream so every matmul gates on exactly one DMA
# semaphore (instructions hold a single wait slot).
WVA = 0
XTA = F                  # xt tile-pairs 0,1 (256 cols)
WVB = XTA + 2 * 128      # 320
XTB = WVB + F            # xt tile-pairs 2,3
X0 = XTB + 2 * 128       # 640: x tiles 0-7 (512 cols)
IN_W = X0 + NT * F       # 1152

AF = mybir.ActivationFunctionType
ALU = mybir.AluOpType

_cache: dict = {}

_ACT_SET = "natural_log_exp_and_others"


def _patch_act_tables():
    """bacc's table chooser greedily picks the lowest-index set containing
    each activation function, which splits {Abs, Exp} and {Ln} across two
    table loads (~2.7us each on the critical path).  Blank every set except
    the one that contains all of Abs/Exp/Ln/Copy so a single load is chosen.
    Positions are preserved, so the emitted act_func_set_id stays valid for
    walrus's lower_act."""
    if _cache.get("act_patched"):
        return
    real = bacc.get_activation_tables

    def only_full_set(arch):
        t = real(arch)
        if _ACT_SET in t:
            t = {k: (v if k == _ACT_SET else set()) for k, v in t.items()}
        return t

    bacc.get_activation_tables = only_full_set
    _cache["act_patched"] = True


def _build_bass():
    _patch_act_tables()
    nc = bacc.Bacc("TRN2", num_devices=N_CORES, enable_partition_id=False)
    in_d = nc.dram_tensor(
        "in_pack", (128, IN_W), mybir.dt.float32, kind="ExternalInput"
    ).ap()
    o_d = nc.dram_tensor(
        "out", (128, NT * F), mybir.dt.float32, kind="ExternalOutput"
    ).ap()


    with tile.TileContext(nc) as tc:
        with (
            tc.tile_pool(name="const", bufs=1) as cpool,
            tc.tile_pool(name="sb", bufs=1) as sb,
            tc.tile_pool(name="ew", bufs=2) as ew,
            tc.tile_pool(name="ps", bufs=1, space="PSUM") as ps,
        ):
            half = cpool.tile([128, 1], mybir.dt.float32)
            nc.gpsimd.memset(half[:], 0.5)

            o_sb = sb.tile([128, NT * F], mybir.dt.float32)
            in_sb = sb.tile([128, IN_W], mybir.dt.float32)
            # Four submits across BOTH HWDGE engines (Sync + Scalar) so two
            # hardware queues stream in parallel; Wv is duplicated into each
            # queue's stream and the splits sit on tile-pair boundaries, so
            # Tile's subtile deps gate each matmul on exactly one DMA.
            P1 = XTA + 128  # wvA + xt pair 0 (chunk 0's sync half)
            P3 = XTB + 128  # wvB + xt pair 1 (chunk 0's scalar half)
            nc.sync.dma_start(in_sb[:, WVA:P1], in_d[:, WVA:P1])
            nc.scalar.dma_start(in_sb[:, WVB:P3], in_d[:, WVB:P3])
            nc.sync.dma_start(in_sb[:, P1:WVB], in_d[:, P1:WVB])
            nc.scalar.dma_start(in_sb[:, P3:X0], in_d[:, P3:X0])
            nc.sync.dma_start(in_sb[:, X0 : X0 + 4 * F], in_d[:, X0 : X0 + 4 * F])
            nc.scalar.dma_start(in_sb[:, X0 + 4 * F :], in_d[:, X0 + 4 * F :])

            # One PSUM tile where each node-tile's matmul owns a full bank
            # (two accumulation groups in one bank hang the PE), while a
            # single strided AP spans several banks for elementwise reads.
            y_full = ps.tile([128, NT, 512], mybir.dt.float32)
            y_ps = y_full[:, :, 0:F]

            bounds = [(i * TPC, (i + 1) * TPC) for i in range(NCHUNK)]
            for lo_t, hi_t in bounds:
                TC_ = hi_t - lo_t
                CW = TC_ * F
                C0 = lo_t * F
                for t in range(lo_t, hi_t):
                    i, h = t // 2, t % 2
                    # pairs 0,2 ride the Sync queue (wvA side); 1,3 Scalar:
                    # each queue's FIRST DMA carries one pair of chunk 0.
                    xt_base = (XTA if i % 2 == 0 else XTB) + (i // 2) * 128
                    wv_base = WVA if i % 2 == 0 else WVB
                    nc.tensor.matmul(
                        y_ps[:, t],
                        in_sb[h * F : (h + 1) * F, xt_base : xt_base + 128],
                        in_sb[h * F : (h + 1) * F, wv_base : wv_base + F],
                        start=True,
                        stop=True,
                    )

                yc = y_ps[:, lo_t:hi_t]                        # [128, TC_, F]
                xc = in_sb[:, X0 + C0 : X0 + C0 + CW].rearrange(
                    "p (t c) -> p t c", t=TC_
                )
                oc = o_sb[:, C0 : C0 + CW].rearrange("p (t c) -> p t c", t=TC_)

                # With g = ln(0.5*e^{-y} + 0.5) (finite for |y| < 88)
                # and s = sign(y):
                #   y >= 0: out = x + y + g;   y < 0: out = x - g
                # i.e. out = (x - g) + (s+1)*(0.5*y + g), exact at y=0 too.
                # Exp reads y straight from PSUM (scale=-1); ACT also takes
                # Sign so the DVE stream stays short.
                e = ew.tile([128, CW], mybir.dt.float32)
                nc.scalar.activation(
                    e[:].rearrange("p (t c) -> p t c", t=TC_), yc, AF.Exp,
                    scale=-1.0,
                )
                g = ew.tile([128, CW], mybir.dt.float32)
                nc.scalar.activation(g[:], e[:], AF.Ln, bias=half[:], scale=0.5)
                s2 = ew.tile([128, CW], mybir.dt.float32)
                nc.scalar.activation(
                    s2[:].rearrange("p (t c) -> p t c", t=TC_), yc, AF.Sign
                )
                t1 = ew.tile([128, CW], mybir.dt.float32)
                nc.vector.tensor_tensor(
                    t1[:].rearrange("p (t c) -> p t c", t=TC_), xc,
                    g[:].rearrange("p (t c) -> p t c", t=TC_), ALU.subtract,
                )
                t2 = ew.tile([128, CW], mybir.dt.float32)
                nc.vector.scalar_tensor_tensor(
                    t2[:].rearrange("p (t c) -> p t c", t=TC_),
                    yc, 0.5, g[:].rearrange("p (t c) -> p t c", t=TC_),
                    ALU.mult, ALU.add,
                )
                t3 = ew.tile([128, CW], mybir.dt.float32)
                nc.vector.scalar_tensor_tensor(
                    t3[:], s2[:], 1.0, t2[:], ALU.add, ALU.mult
                )
                nc.vector.tensor_tensor(oc, t1[:].rearrange("p (t c) -> p t c", t=TC_), t3[:].rearrange("p (t c) -> p t c", t=TC_), ALU.add)
                # Two output submits per chunk, one per HWDGE engine, so both
                # queues drain the result in parallel.
                HW = CW // 2
                for h2 in range(2):
                    eng = nc.sync if h2 == 0 else nc.scalar
                    lo = C0 + h2 * HW
                    eng.dma_start(o_d[:, lo : lo + HW], o_sb[:, lo : lo + HW])
    nc.compile()
    return nc


def _prep_core_inputs(x, Wv):
    """Pack each core's inputs into one [128, 1152] tensor:
    [wvA | xt01 | wvB | xt23 | x], matching the WVA/XTA/WVB/XTB/X0 layout.

    wv2: Wv stacked twice (h0/h64 matmul pairs need rhs at both base
    partitions); it appears once per HWDGE queue stream.
    xt[h*64+ch, 128*i + p] = xs[(2i+h)*128 + p, ch].
    xp[p, t*64+c] = xs[t*128 + p, c] (the SBUF node-tile layout).
    """
    wv2 = np.vstack([Wv, Wv])  # [128, F]
    maps = []
    for i in range(N_CORES):
        xs = x[i * NS : (i + 1) * NS]
        xt = xs.reshape(NT // 2, 2, 128, F).transpose(1, 3, 0, 2).reshape(128, NT * F)
        xp = xs.reshape(NT, 128, F).transpose(1, 0, 2).reshape(128, NT * F)
        maps.append(
            {
                "in_pack": np.ascontiguousarray(
                    np.hstack(
                        [
                            wv2, xt[:, 0:128], xt[:, 256:384],
                            wv2, xt[:, 128:256], xt[:, 384:512], xp,
                        ]
                    )
                )
            }
        )
    return maps


def _run(x, Wv, edges_dst, trace=False):
    x = np.ascontiguousarray(np.asarray(x, dtype=np.float32))
    Wv = np.ascontiguousarray(np.asarray(Wv, dtype=np.float32))
    if "nc" not in _cache:
        _cache["nc"] = _build_bass()
    nc = _cache["nc"]
    in_maps = _prep_core_inputs(x, Wv)
    res = run_bass_kernel_spmd(
        nc, in_maps, core_ids=list(range(N_CORES)), trace=trace
    )
    # Unpack [128, NT*F] node-tile layout back to [NS, F] per core.
    out = np.concatenate(
        [
            r["out"].reshape(128, NT, F).transpose(1, 0, 2).reshape(NS, F)
            for r in res.results
        ],
        axis=0,
    )
    # Residual-only rows: nodes with no incoming edge keep x unchanged.
    indeg = np.bincount(
        np.asarray(edges_dst).astype(np.int64), minlength=N_NODES
    )[:N_NODES]
    dead = indeg == 0
    if dead.any():
        out[dead] = x[dead]
    return out, res


def kernel(x, Wq, Wk, Wv, Wqk, edges_src, edges_dst):
    out, _ = _run(x, Wv, edges_dst)
    return out



# revision 58
# speedup vs baseline: 1.0454x; 1.0178x over previous
"""Trainium2 Bass kernel for nn_AttentionBlock_38225208934579.

The reference attention block collapses algebraically: the scatter-sum
gathers v at edges_dst and scatters back to edges_dst, so for every
destination node d the attention weights (which sum to 1 over d's
segment) multiply the same vector v[d]:

    out[d] = x[d] + v[d] * [indegree(d) > 0],   v = norm_act(x @ Wv)

norm_act over 64x0e scalars is elementwise; with y = x @ Wv, u = |y|:

    v = sign(y) * (softplus(u) - log2)
      = y + w * sign(y),   w = ln(0.5*e^{-u} + 0.5)   (w in [-log2, 0])

so out = (x + y) + w*sign(y) — no division, no reciprocal needed.
The q/k/Wqk path of the reference is dead code.

Sharding: data parallel over nodes — each of the 8 cores handles 1024
nodes (8 graphs); the FxF weight is replicated.  Host-side prep packs
everything one core needs into a single [128, 1152] tensor (Wv once per
DMA-queue stream, a pair-interleaved transposed x so PE matmuls run as
h0/h64 row-group pairs with no on-device transposes, and x in SBUF
node-tile layout);
device output is the packed [128, 512] node-tile layout, unpacked on
the host.  Zero-indegree nodes (impossible for this problem's
block-diagonal fully-connected edges, where every node has 128
in-edges) keep x unchanged and are fixed up on the host.
"""

import math

import numpy as np

import concourse.mybir as mybir
import concourse.tile as tile
from concourse import bacc
from concourse.bass_utils import run_bass_kernel_spmd

N_NODES = 8192
F = 64
N_CORES = 8
NS = N_NODES // N_CORES  # 1024 nodes per core
NT = NS // 128           # 8 node-tiles of 128 per core
import os as _os
NCHUNK = int(_os.environ.get("K_NCHUNK", "2"))  # pipeline chunks (must be 2 or 4)
TPC = NT // NCHUNK       # tiles per chunk (4)
# Packed-input layout: [wvA | xt01 | wvB | xt23 | x03 | x47].  Wv appears
# once per HWDGE queue stream so every matmul gates on exactly one DMA
# semaphore (instructions hold a single wait slot).
WVA = 0
XTA = F                  # xt tile-pairs 0,1 (256 cols)
WVB = XTA + 2 * 128      # 320
XTB = WVB + F            # xt tile-pairs 2,3
X0 = XTB + 2 * 128       # 640: x tiles 0-7 (512 cols)
IN_W = X0 + NT * F       # 1152

AF = mybir.ActivationFunctionType
ALU = mybir.AluOpType

_cache: dict = {}

_ACT_SET = "natural_log_exp_and_others"


def _patch_act_tables():
    """bacc's table chooser greedily picks the lowest-index set containing
    each activation function, which splits {Abs, Exp} and {Ln} across two
    table loads (~2.7us each on the critical path).  Blank every set except
    the one that contains all of Abs/Exp/Ln/Copy so a single load is chosen.
    Positions are preserved, so the emitted act_func_set_id stays valid for
    walrus's lower_act."""
    if _cache.get("act_patched"):
        return
    real = bacc.get_activation_tables

    def only_full_set(arch):
        t = real(arch)
        if _ACT_SET in t:
            t = {k: (v if k == _ACT_SET else set()) for k, v in t.items()}
        return t

    bacc.get_activation_tables = only_full_set
    _cache["act_patched"] = True


def _build_bass():
    _patch_act_tables()
    nc = bacc.Bacc("TRN2", num_devices=N_CORES, enable_partition_id=False)
    in_d = nc.dram_tensor(
        "in_pack", (128, IN_W), mybir.dt.float32, kind="ExternalInput"
    ).ap()
    o_d = nc.dram_tensor(
        "out", (128, NT * F), mybir.dt.float32, kind="ExternalOutput"
    ).ap()


    with tile.TileContext(nc) as tc:
        with (
            tc.tile_pool(name="const", bufs=1) as cpool,
            tc.tile_pool(name="sb", bufs=1) as sb,
            tc.tile_pool(name="ew", bufs=2) as ew,
            tc.tile_pool(name="ps", bufs=1, space="PSUM") as ps,
        ):
            half = cpool.tile([128, 1], mybir.dt.float32)
            nc.gpsimd.memset(half[:], 0.5)

            o_sb = sb.tile([128, NT * F], mybir.dt.float32)
            in_sb = sb.tile([128, IN_W], mybir.dt.float32)
            # Four submits across BOTH HWDGE engines (Sync + Scalar) so two
            # hardware queues stream in parallel; Wv is duplicated into each
            # queue's stream and the splits sit on tile-pair boundaries, so
            # Tile's subtile deps gate each matmul on exactly one DMA.
            P1 = XTA + 128  # wvA + xt pair 0 (chunk 0's sync half)
            P3 = XTB + 128  # wvB + xt pair 1 (chunk 0's scalar half)
            nc.sync.dma_start(in_sb[:, WVA:P1], in_d[:, WVA:P1])
            nc.scalar.dma_start(in_sb[:, WVB:P3], in_d[:, WVB:P3])
            nc.sync.dma_start(in_sb[:, P1:WVB], in_d[:, P1:WVB])
            nc.scalar.dma_start(in_sb[:, P3:X0], in_d[:, P3:X0])
            nc.sync.dma_start(in_sb[:, X0 : X0 + 4 * F], in_d[:, X0 : X0 + 4 * F])
            nc.scalar.dma_start(in_sb[:, X0 + 4 * F :], in_d[:, X0 + 4 * F :])

            # One PSUM tile where each node-tile's matmul owns a full bank
            # (two accumulation groups in one bank hang the PE), while a
            # single strided AP spans several banks for elementwise reads.
            y_full = ps.tile([128, NT, 512], mybir.dt.float32)
            y_ps = y_full[:, :, 0:F]

            bounds = [(i * TPC, (i + 1) * TPC) for i in range(NCHUNK)]
            for lo_t, hi_t in bounds:
                TC_ = hi_t - lo_t
                CW = TC_ * F
                C0 = lo_t * F
                for t in range(lo_t, hi_t):
                    i, h = t // 2, t % 2
                    # pairs 0,2 ride the Sync queue (wvA side); 1,3 Scalar:
                    # each queue's FIRST DMA carries one pair of chunk 0.
                    xt_base = (XTA if i % 2 == 0 else XTB) + (i // 2) * 128
                    wv_base = WVA if i % 2 == 0 else WVB
                    nc.tensor.matmul(
                        y_ps[:, t],
                        in_sb[h * F : (h + 1) * F, xt_base : xt_base + 128],
                        in_sb[h * F : (h + 1) * F, wv_base : wv_base + F],
                        start=True,
                        stop=True,
                    )

                yc = y_ps[:, lo_t:hi_t]                        # [128, TC_, F]
                xc = in_sb[:, X0 + C0 : X0 + C0 + CW].rearrange(
                    "p (t c) -> p t c", t=TC_
                )
                oc = o_sb[:, C0 : C0 + CW].rearrange("p (t c) -> p t c", t=TC_)

                # With g = ln(0.5*e^{-y} + 0.5) (finite for |y| < 88)
                # and s = sign(y):
                #   y >= 0: out = x + y + g;   y < 0: out = x - g
                # i.e. out = (x - g) + (s+1)*(0.5*y + g), exact at y=0 too.
                # Exp reads y straight from PSUM (scale=-1); ACT also takes
                # Sign so the DVE stream stays short.
                e = ew.tile([128, CW], mybir.dt.float32)
                nc.scalar.activation(
                    e[:].rearrange("p (t c) -> p t c", t=TC_), yc, AF.Exp,
                    scale=-1.0,
                )
                g = ew.tile([128, CW], mybir.dt.float32)
                nc.scalar.activation(g[:], e[:], AF.Ln, bias=half[:], scale=0.5)
                s2 = ew.tile([128, CW], mybir.dt.float32)
                nc.scalar.activation(
                    s2[:].rearrange("p (t c) -> p t c", t=TC_), yc, AF.Sign
                )
                t1 = ew.tile([128, CW], mybir.dt.float32)
                # x - g is off the critical path (only the final add needs
                # it): run it on the otherwise-idle GPSIMD to shorten the
                # DVE stream.
                nc.gpsimd.tensor_tensor(
                    t1[:].rearrange("p (t c) -> p t c", t=TC_), xc,
                    g[:].rearrange("p (t c) -> p t c", t=TC_), ALU.subtract,
                )
                t2 = ew.tile([128, CW], mybir.dt.float32)
                nc.vector.scalar_tensor_tensor(
                    t2[:].rearrange("p (t c) -> p t c", t=TC_),
                    yc, 0.5, g[:].rearrange("p (t c) -> p t c", t=TC_),
                    ALU.mult, ALU.add,
                )
                t3 = ew.tile([128, CW], mybir.dt.float32)
                nc.vector.scalar_tensor_tensor(
                    t3[:], s2[:], 1.0, t2[:], ALU.add, ALU.mult
                )
                nc.vector.tensor_tensor(oc, t1[:].rearrange("p (t c) -> p t c", t=TC_), t3[:].rearrange("p (t c) -> p t c", t=TC_), ALU.add)
                # Two output submits per chunk, one per HWDGE engine, so both
                # queues drain the result in parallel.
                HW = CW // 2
                for h2 in range(2):
                    eng = nc.sync if h2 == 0 else nc.scalar
                    lo = C0 + h2 * HW
                    eng.dma_start(o_d[:, lo : lo + HW], o_sb[:, lo : lo + HW])
    nc.compile()
    return nc


def _prep_core_inputs(x, Wv):
    """Pack each core's inputs into one [128, 1152] tensor:
    [wvA | xt01 | wvB | xt23 | x], matching the WVA/XTA/WVB/XTB/X0 layout.

    wv2: Wv stacked twice (h0/h64 matmul pairs need rhs at both base
    partitions); it appears once per HWDGE queue stream.
    xt[h*64+ch, 128*i + p] = xs[(2i+h)*128 + p, ch].
    xp[p, t*64+c] = xs[t*128 + p, c] (the SBUF node-tile layout).
    """
    wv2 = np.vstack([Wv, Wv])  # [128, F]
    maps = []
    for i in range(N_CORES):
        xs = x[i * NS : (i + 1) * NS]
        xt = xs.reshape(NT // 2, 2, 128, F).transpose(1, 3, 0, 2).reshape(128, NT * F)
        xp = xs.reshape(NT, 128, F).transpose(1, 0, 2).reshape(128, NT * F)
        maps.append(
            {
                "in_pack": np.ascontiguousarray(
                    np.hstack(
                        [
                            wv2, xt[:, 0:128], xt[:, 256:384],
                            wv2, xt[:, 128:256], xt[:, 384:512], xp,
                        ]
                    )
                )
            }
        )
    return maps


def _run(x, Wv, edges_dst, trace=False):
    x = np.ascontiguousarray(np.asarray(x, dtype=np.float32))
    Wv = np.ascontiguousarray(np.asarray(Wv, dtype=np.float32))
    if "nc" not in _cache:
        _cache["nc"] = _build_bass()
    nc = _cache["nc"]
    in_maps = _prep_core_inputs(x, Wv)
    res = run_bass_kernel_spmd(
        nc, in_maps, core_ids=list(range(N_CORES)), trace=trace
    )
    # Unpack [128, NT*F] node-tile layout back to [NS, F] per core.
    out = np.concatenate(
        [
            r["out"].reshape(128, NT, F).transpose(1, 0, 2).reshape(NS, F)
            for r in res.results
        ],
        axis=0,
    )
    # Residual-only rows: nodes with no incoming edge keep x unchanged.
    indeg = np.bincount(
        np.asarray(edges_dst).astype(np.int64), minlength=N_NODES
    )[:N_NODES]
    dead = indeg == 0
    if dead.any():
        out[dead] = x[dead]
    return out, res


def kernel(x, Wq, Wk, Wv, Wqk, edges_src, edges_dst):
    out, _ = _run(x, Wv, edges_dst)
    return out



# revision 60
# speedup vs baseline: 1.1169x; 1.0684x over previous
"""Trainium2 Bass kernel for nn_AttentionBlock_38225208934579.

The reference attention block collapses algebraically: the scatter-sum
gathers v at edges_dst and scatters back to edges_dst, so for every
destination node d the attention weights (which sum to 1 over d's
segment) multiply the same vector v[d]:

    out[d] = x[d] + v[d] * [indegree(d) > 0],   v = norm_act(x @ Wv)

norm_act over 64x0e scalars is elementwise; with y = x @ Wv, u = |y|:

    v = sign(y) * (softplus(u) - log2)
      = y + w * sign(y),   w = ln(0.5*e^{-u} + 0.5)   (w in [-log2, 0])

so out = (x + y) + w*sign(y) — no division, no reciprocal needed.
The q/k/Wqk path of the reference is dead code.

Sharding: data parallel over nodes — each of the 8 cores handles 1024
nodes (8 graphs); the FxF weight is replicated.  Host-side prep packs
everything one core needs into a single [128, 1152] tensor (Wv once per
DMA-queue stream, a pair-interleaved transposed x so PE matmuls run as
h0/h64 row-group pairs with no on-device transposes, and x in SBUF
node-tile layout);
device output is the packed [128, 512] node-tile layout, unpacked on
the host.  Zero-indegree nodes (impossible for this problem's
block-diagonal fully-connected edges, where every node has 128
in-edges) keep x unchanged and are fixed up on the host.
"""

import math

import numpy as np

import concourse.mybir as mybir
import concourse.tile as tile
from concourse import bacc, dve_ops
from concourse.bass_utils import run_bass_kernel_spmd
from concourse.dve_spec import Spec, Src0, Src1, Zero, lower, select
from concourse.dve_table_gen import dve_ver_for
from concourse.dve_uop import DveOpSpec


def _register_sel_op():
    """Custom DVE op: out = select(in1 >= 0, 2*in0, 0) — replaces the
    ACT Sign + fused multiply pair (t2 is exactly 0 at y == 0, so the
    doubled positive branch is correct there too)."""
    for op in dve_ops.OPS:
        if op.name == "SEL_POS_DBL_ANT":
            return op
    spec = Spec(
        body=select(Src1 >= Zero, Src0 + Src0, Zero),
        reference=lambda in0, in1, s0, s1, imm2: np.where(
            np.asarray(in1).reshape(np.asarray(in0).shape) >= 0, in0 + in0, 0.0
        ).astype(np.float32),
    )
    name = "SEL_POS_DBL_ANT"
    row = max(dve_ops._SUB_OPCODE_FOR_NAME.values()) + 1
    assert row < 0x20
    dve_ops._SUB_OPCODE_FOR_NAME[name] = row
    ver = dve_ver_for("TRN2")
    sha = DveOpSpec(
        name=name, opcode=row, uops=lower(spec, ver=ver), rd1_en=True
    ).sha(ver)
    op = dve_ops.DveOp(name, spec, subdim=False, uops_sha={ver: sha})
    dve_ops.OPS.append(op)
    dve_ops.CUSTOM_DVE_SPECS[name] = spec
    return op

N_NODES = 8192
F = 64
N_CORES = 8
NS = N_NODES // N_CORES  # 1024 nodes per core
NT = NS // 128           # 8 node-tiles of 128 per core
import os as _os
NCHUNK = int(_os.environ.get("K_NCHUNK", "2"))  # pipeline chunks (must be 2 or 4)
TPC = NT // NCHUNK       # tiles per chunk (4)
# Packed-input layout: [wvA | xt01 | wvB | xt23 | x03 | x47].  Wv appears
# once per HWDGE queue stream so every matmul gates on exactly one DMA
# semaphore (instructions hold a single wait slot).
WVA = 0
XTA = F                  # xt tile-pairs 0,1 (256 cols)
WVB = XTA + 2 * 128      # 320
XTB = WVB + F            # xt tile-pairs 2,3
X0 = XTB + 2 * 128       # 640: x tiles 0-7 (512 cols)
IN_W = X0 + NT * F       # 1152

AF = mybir.ActivationFunctionType
ALU = mybir.AluOpType

_cache: dict = {}

_ACT_SET = "natural_log_exp_and_others"


def _patch_act_tables():
    """bacc's table chooser greedily picks the lowest-index set containing
    each activation function, which splits {Abs, Exp} and {Ln} across two
    table loads (~2.7us each on the critical path).  Blank every set except
    the one that contains all of Abs/Exp/Ln/Copy so a single load is chosen.
    Positions are preserved, so the emitted act_func_set_id stays valid for
    walrus's lower_act."""
    if _cache.get("act_patched"):
        return
    real = bacc.get_activation_tables

    def only_full_set(arch):
        t = real(arch)
        if _ACT_SET in t:
            t = {k: (v if k == _ACT_SET else set()) for k, v in t.items()}
        return t

    bacc.get_activation_tables = only_full_set
    _cache["act_patched"] = True


def _build_bass():
    _patch_act_tables()
    sel_op = _register_sel_op()
    nc = bacc.Bacc("TRN2", num_devices=N_CORES, enable_partition_id=False)
    in_d = nc.dram_tensor(
        "in_pack", (128, IN_W), mybir.dt.float32, kind="ExternalInput"
    ).ap()
    o_d = nc.dram_tensor(
        "out", (128, NT * F), mybir.dt.float32, kind="ExternalOutput"
    ).ap()


    with tile.TileContext(nc) as tc:
        with (
            tc.tile_pool(name="const", bufs=1) as cpool,
            tc.tile_pool(name="sb", bufs=1) as sb,
            tc.tile_pool(name="ew", bufs=2) as ew,
            tc.tile_pool(name="ps", bufs=1, space="PSUM") as ps,
        ):
            half = cpool.tile([128, 1], mybir.dt.float32)
            nc.gpsimd.memset(half[:], 0.5)

            o_sb = sb.tile([128, NT * F], mybir.dt.float32)
            in_sb = sb.tile([128, IN_W], mybir.dt.float32)
            # Four submits across BOTH HWDGE engines (Sync + Scalar) so two
            # hardware queues stream in parallel; Wv is duplicated into each
            # queue's stream and the splits sit on tile-pair boundaries, so
            # Tile's subtile deps gate each matmul on exactly one DMA.
            P1 = XTA + 128  # wvA + xt pair 0 (chunk 0's sync half)
            P3 = XTB + 128  # wvB + xt pair 1 (chunk 0's scalar half)
            nc.sync.dma_start(in_sb[:, WVA:P1], in_d[:, WVA:P1])
            nc.scalar.dma_start(in_sb[:, WVB:P3], in_d[:, WVB:P3])
            nc.sync.dma_start(in_sb[:, P1:WVB], in_d[:, P1:WVB])
            nc.scalar.dma_start(in_sb[:, P3:X0], in_d[:, P3:X0])
            nc.sync.dma_start(in_sb[:, X0 : X0 + 4 * F], in_d[:, X0 : X0 + 4 * F])
            nc.scalar.dma_start(in_sb[:, X0 + 4 * F :], in_d[:, X0 + 4 * F :])

            # One PSUM tile where each node-tile's matmul owns a full bank
            # (two accumulation groups in one bank hang the PE), while a
            # single strided AP spans several banks for elementwise reads.
            y_full = ps.tile([128, NT, 512], mybir.dt.float32)
            y_ps = y_full[:, :, 0:F]

            bounds = [(i * TPC, (i + 1) * TPC) for i in range(NCHUNK)]
            for lo_t, hi_t in bounds:
                TC_ = hi_t - lo_t
                CW = TC_ * F
                C0 = lo_t * F
                for t in range(lo_t, hi_t):
                    i, h = t // 2, t % 2
                    # pairs 0,2 ride the Sync queue (wvA side); 1,3 Scalar:
                    # each queue's FIRST DMA carries one pair of chunk 0.
                    xt_base = (XTA if i % 2 == 0 else XTB) + (i // 2) * 128
                    wv_base = WVA if i % 2 == 0 else WVB
                    nc.tensor.matmul(
                        y_ps[:, t],
                        in_sb[h * F : (h + 1) * F, xt_base : xt_base + 128],
                        in_sb[h * F : (h + 1) * F, wv_base : wv_base + F],
                        start=True,
                        stop=True,
                    )

                yc = y_ps[:, lo_t:hi_t]                        # [128, TC_, F]
                xc = in_sb[:, X0 + C0 : X0 + C0 + CW].rearrange(
                    "p (t c) -> p t c", t=TC_
                )
                oc = o_sb[:, C0 : C0 + CW].rearrange("p (t c) -> p t c", t=TC_)

                # With g = ln(0.5*e^{-y} + 0.5) (finite for |y| < 88)
                # and s = sign(y):
                #   y >= 0: out = x + y + g;   y < 0: out = x - g
                # i.e. out = (x - g) + (s+1)*(0.5*y + g), exact at y=0 too.
                # Exp reads y straight from PSUM (scale=-1); ACT also takes
                # Sign so the DVE stream stays short.
                e = ew.tile([128, CW], mybir.dt.float32)
                nc.scalar.activation(
                    e[:].rearrange("p (t c) -> p t c", t=TC_), yc, AF.Exp,
                    scale=-1.0,
                )
                g = ew.tile([128, CW], mybir.dt.float32)
                nc.scalar.activation(g[:], e[:], AF.Ln, bias=half[:], scale=0.5)
                t1 = ew.tile([128, CW], mybir.dt.float32)
                # x - g is off the critical path (only the final add needs
                # it): run it on the otherwise-idle GPSIMD to shorten the
                # DVE stream.
                nc.gpsimd.tensor_tensor(
                    t1[:].rearrange("p (t c) -> p t c", t=TC_), xc,
                    g[:].rearrange("p (t c) -> p t c", t=TC_), ALU.subtract,
                )
                t2 = ew.tile([128, CW], mybir.dt.float32)
                nc.vector.scalar_tensor_tensor(
                    t2[:].rearrange("p (t c) -> p t c", t=TC_),
                    yc, 0.5, g[:].rearrange("p (t c) -> p t c", t=TC_),
                    ALU.mult, ALU.add,
                )
                t3 = ew.tile([128, CW], mybir.dt.float32)
                nc.vector._custom_dve(
                    sel_op,
                    out=t3[:].rearrange("p (t c) -> p t c", t=TC_),
                    in0=t2[:].rearrange("p (t c) -> p t c", t=TC_),
                    in1=yc,
                )
                nc.vector.tensor_tensor(oc, t1[:].rearrange("p (t c) -> p t c", t=TC_), t3[:].rearrange("p (t c) -> p t c", t=TC_), ALU.add)
                # Two output submits per chunk, one per HWDGE engine, so both
                # queues drain the result in parallel.
                HW = CW // 2
                for h2 in range(2):
                    eng = nc.sync if h2 == 0 else nc.scalar
                    lo = C0 + h2 * HW
                    eng.dma_start(o_d[:, lo : lo + HW], o_sb[:, lo : lo + HW])
    nc.compile()
    return nc


def _prep_core_inputs(x, Wv):
    """Pack each core's inputs into one [128, 1152] tensor:
    [wvA | xt01 | wvB | xt23 | x], matching the WVA/XTA/WVB/XTB/X0 layout.

    wv2: Wv stacked twice (h0/h64 matmul pairs need rhs at both base
    partitions); it appears once per HWDGE queue stream.
    xt[h*64+ch, 128*i + p] = xs[(2i+h)*128 + p, ch].
    xp[p, t*64+c] = xs[t*128 + p, c] (the SBUF node-tile layout).
    """
    wv2 = np.vstack([Wv, Wv])  # [128, F]
    maps = []
    for i in range(N_CORES):
        xs = x[i * NS : (i + 1) * NS]
        xt = xs.reshape(NT // 2, 2, 128, F).transpose(1, 3, 0, 2).reshape(128, NT * F)
        xp = xs.reshape(NT, 128, F).transpose(1, 0, 2).reshape(128, NT * F)
        maps.append(
            {
                "in_pack": np.ascontiguousarray(
                    np.hstack(
                        [
                            wv2, xt[:, 0:128], xt[:, 256:384],
                            wv2, xt[:, 128:256], xt[:, 384:512], xp,
                        ]
                    )
                )
            }
        )
    return maps


def _run(x, Wv, edges_dst, trace=False):
    x = np.ascontiguousarray(np.asarray(x, dtype=np.float32))
    Wv = np.ascontiguousarray(np.asarray(Wv, dtype=np.float32))
    if "nc" not in _cache:
        _cache["nc"] = _build_bass()
    nc = _cache["nc"]
    in_maps = _prep_core_inputs(x, Wv)
    res = run_bass_kernel_spmd(
        nc, in_maps, core_ids=list(range(N_CORES)), trace=trace
    )
    # Unpack [128, NT*F] node-tile layout back to [NS, F] per core.
    out = np.concatenate(
        [
            r["out"].reshape(128, NT, F).transpose(1, 0, 2).reshape(NS, F)
            for r in res.results
        ],
        axis=0,
    )
    # Residual-only rows: nodes with no incoming edge keep x unchanged.
    indeg = np.bincount(
        np.asarray(edges_dst).astype(np.int64), minlength=N_NODES
    )[:N_NODES]
    dead = indeg == 0
    if dead.any():
        out[dead] = x[dead]
    return out, res


def kernel(x, Wq, Wk, Wv, Wqk, edges_src, edges_dst):
    out, _ = _run(x, Wv, edges_dst)
    return out

